# revision 26
# baseline (speedup 1.0000x reference)
"""Multi-head attention + output projection on 8 Trainium2 NeuronCores.

Problem (hardcoded): B=2, N=S=2048, DIM=1024, 8 heads, head_dim=128, fp32.
  out = softmax(Q K^T / sqrt(128)) V  -> reshape -> @ proj_w.T + proj_b

Sharding: data parallel on batch (2) x tensor parallel on heads (4 groups of
2 heads).  Each core computes attention for its 2 heads plus the partial
output projection restricted to its heads' columns; the host sums the 4
partial projections per batch and adds the bias.

Per-core kernel (matmul operands fp16, accumulation fp32 PSUM):
  S^T = K @ Q^T per 128-row s-chunk with s on partitions (softmax needs no
  on-chip transpose of P); exp on ScalarE (PSUM->SBUF, scale pre-applied to
  Q on host); out^T = V^T @ expS^T accumulated in PSUM.  Row sums feed an
  all-ones [128x128] matmul that colsums over partitions with the result
  broadcast to all 128 rows; reciprocal_approx_fast + multiply normalizes.

  The emission order is software-pipelined for the in-order engine queues:
  PV lags one group behind QK/exp, and each head's tail work (last PV,
  rowsum-tail matmuls, reciprocal, normalize) plus each block's projection
  are deferred into the NEXT head's stream so the exp pipeline on ScalarE
  never drains at head boundaries.
"""

import sys

sys.path.insert(0, "/opt/trn_rl_repo")

import numpy as np

import concourse.bass as bass  # noqa: F401  (engine namespaces live on nc)
import concourse.mybir as mybir
import concourse.tile as tile
from concourse import bacc
from concourse.bass_utils import run_bass_kernel_spmd

B = 2
N = 2048
S = 2048
DIM = 1024
NUM_HEADS = 8
HD = 128
N_CORES = 8
HEADS_PER_CORE = 2  # 4-way head parallel x 2-way batch parallel
HG = DIM // (NUM_HEADS // HEADS_PER_CORE)  # 256 dims per core
P = 128
SC = S // P  # 16 s-chunks
NB = 512  # query-column block
NQ = N // NB
GC = 2  # s-chunks per exp group
NG = SC // GC  # 8 groups per (head, block)
F32 = mybir.dt.float32
F16 = mybir.dt.float16

_nc_cache = {}


def _build():
    nc = bacc.Bacc(None, target_bir_lowering=False, debug=False, num_devices=1)

    qt = nc.dram_tensor("qt", [HG, N], F16, kind="ExternalInput").ap()
    kt = nc.dram_tensor("kt", [HG, S], F16, kind="ExternalInput").ap()
    v = nc.dram_tensor("v", [S, HG], F16, kind="ExternalInput").ap()
    wt = nc.dram_tensor("wt", [HG, DIM], F16, kind="ExternalInput").ap()
    out = nc.dram_tensor("out", [N, DIM], F32, kind="ExternalOutput").ap()

    EXPF = mybir.ActivationFunctionType.Exp

    with tile.TileContext(nc) as tc:
        with (
            tc.tile_pool(name="persist", bufs=1) as persist,
            tc.tile_pool(name="e_pool", bufs=5) as e_pool,
            tc.tile_pool(name="a_pool", bufs=2) as a_pool,
            tc.tile_pool(name="small", bufs=2) as small,
            tc.tile_pool(name="y_pool", bufs=2) as y_pool,
            tc.tile_pool(name="s_ps_pool", bufs=2, space="PSUM") as s_ps_pool,
            tc.tile_pool(name="acc_ps_pool", bufs=4, space="PSUM") as acc_ps_pool,
        ):
            # Resident inputs, sliced so the first QK group starts early.
            qt_sb = persist.tile([P, HEADS_PER_CORE, N], F16)
            kt_sb = persist.tile([P, HEADS_PER_CORE, S], F16)
            v_sb = persist.tile([P, HEADS_PER_CORE, SC, HD], F16)
            wt_sb = persist.tile([P, HEADS_PER_CORE, DIM], F16)
            qt_r = qt.rearrange("(h p) n -> p h n", p=P)
            kt_r = kt.rearrange("(h p) s -> p h s", p=P)
            v_r = v.rearrange("(c p) (h d) -> p h c d", p=P, h=HEADS_PER_CORE)
            wt_r = wt.rearrange("(h p) o -> p h o", p=P)
            ones_dram = nc.inline_tensor(np.ones((P, P), np.float16), name="ones_const")
            ones_mat = persist.tile([P, P], F16)
            nc.sync.dma_start(out=qt_sb[:, 0, 0:NB], in_=qt_r[:, 0, 0:NB])
            nc.sync.dma_start(out=kt_sb[:, 0, 0 : S // 4], in_=kt_r[:, 0, 0 : S // 4])
            nc.sync.dma_start(out=kt_sb[:, 0, S // 4 : S // 2], in_=kt_r[:, 0, S // 4 : S // 2])
            nc.sync.dma_start(out=v_sb[:, 0, 0 : SC // 2], in_=v_r[:, 0, 0 : SC // 2])
            nc.sync.dma_start(out=kt_sb[:, 0, S // 2 :], in_=kt_r[:, 0, S // 2 :])
            nc.sync.dma_start(out=v_sb[:, 0, SC // 2 :], in_=v_r[:, 0, SC // 2 :])
            nc.sync.dma_start(out=ones_mat, in_=ones_dram.ap())
            nc.sync.dma_start(out=qt_sb[:, 0, NB:], in_=qt_r[:, 0, NB:])
            nc.sync.dma_start(out=kt_sb[:, 1], in_=kt_r[:, 1])
            nc.sync.dma_start(out=qt_sb[:, 1], in_=qt_r[:, 1])
            nc.sync.dma_start(out=v_sb[:, 1], in_=v_r[:, 1])
            nc.sync.dma_start(out=wt_sb, in_=wt_r)

            # X^T: normalized attention outputs, head-dim on partitions.
            xt_sb = persist.tile([P, HEADS_PER_CORE, N], F16)

            def make_tail(o_ps, rb_ps, a2, a2g, e6, e7, h, nsl):
                def tail():
                    # last PV pair (group 7 only; group 6 ran in the main loop)
                    for j in range(GC):
                        si = GC * (NG - 1) + j
                        nc.tensor.matmul(
                            o_ps, v_sb[:, h, si, :], e7[:, j, :],
                            start=False, stop=(si == SC - 1),
                        )
                    with nc.allow_low_precision(reason="fp16 rowsum partials"):
                        nc.vector.tensor_add(a2g, a2g, e7)
                        nc.vector.tensor_add(a2, a2, a2g)
                    # rowsum: a2 halves now cover all 16 chunks
                    parts = [a2[:, 0, :], a2[:, 1, :]]
                    for pi, part in enumerate(parts):
                        nc.tensor.matmul(
                            rb_ps, ones_mat, part,
                            start=(pi == 0), stop=(pi == len(parts) - 1),
                        )
                    recip = small.tile([P, NB], F32, tag="recip")
                    nc.vector.reciprocal_approx_fast(out=recip, in_=rb_ps)
                    with nc.allow_low_precision(reason="fp16 attention output grid"):
                        nc.vector.tensor_mul(xt_sb[:, h, nsl], o_ps, recip)
                return tail

            def make_proj(nq, t, y_sb):
                def proj():
                    nt = nq * (NB // P) + t
                    for ot in range(2):
                        y_ps = acc_ps_pool.tile([P, NB], F32, tag="acc")
                        for h in range(HEADS_PER_CORE):
                            nc.tensor.matmul(
                                y_ps,
                                xt_sb[:, h, nt * P : (nt + 1) * P],
                                wt_sb[:, h, ot * NB : (ot + 1) * NB],
                                start=(h == 0),
                                stop=(h == HEADS_PER_CORE - 1),
                            )
                        nc.vector.tensor_copy(y_sb[:, t, ot * NB : (ot + 1) * NB], y_ps)
                    if t == NB // P - 1:
                        nc.sync.dma_start(
                            out=out[nq * NB : (nq + 1) * NB, :].rearrange(
                                "(t p) o -> p t o", p=P
                            ),
                            in_=y_sb,
                        )
                return proj

            pending_tail = None
            pending_proj = []
            for nq in range(NQ):
                nsl = slice(nq * NB, (nq + 1) * NB)
                for h in range(HEADS_PER_CORE):
                    q_blk = qt_sb[:, h, nsl]
                    o_ps = acc_ps_pool.tile([P, NB], F32, tag="acc")
                    a2 = a_pool.tile([P, GC, NB], F16, tag="a2")
                    a2g = a_pool.tile([P, GC, NB], F16, tag="a2g")
                    rb_ps = None
                    es = []  # exp tiles in flight
                    for g in range(NG):
                        s_ps = s_ps_pool.tile([P, GC, NB], F32, tag="s")
                        for j in range(GC):
                            si = GC * g + j
                            nc.tensor.matmul(
                                s_ps[:, j, :],
                                kt_sb[:, h, si * P : (si + 1) * P],
                                q_blk,
                                start=True,
                                stop=True,
                            )
                        e_t = e_pool.tile([P, GC, NB], F16, tag="e")
                        nc.scalar.activation(out=e_t, in_=s_ps, func=EXPF)
                        es.append(e_t)

                        if g == 0 and pending_tail is not None:
                            pending_tail()
                            pending_tail = None
                        if 4 <= g <= 7 and pending_proj:
                            pending_proj.pop(0)()

                        # PV + rowsum accumulation lag one group behind exp
                        if g >= 1:
                            pg = g - 1
                            pe = es[pg]
                            for j in range(GC):
                                si = GC * pg + j
                                nc.tensor.matmul(
                                    o_ps, v_sb[:, h, si, :], pe[:, j, :],
                                    start=(si == 0), stop=False,
                                )
                            with nc.allow_low_precision(
                                reason="fp16 rowsum partials; r ~2e3, ~3e-4 rel"
                            ):
                                if pg in (2, 3):
                                    acc = a2 if pg == 2 else a2g
                                    nc.vector.tensor_add(acc, es[pg - 2], pe)
                                elif pg >= 4:
                                    acc = a2 if pg % 2 == 0 else a2g
                                    nc.vector.tensor_add(acc, acc, pe)
                        if g == NG - 1:
                            rb_ps = acc_ps_pool.tile([P, NB], F32, tag="acc")
                    # defer: PV of group 7 + rowsum tail + normalize
                    pending_tail = make_tail(o_ps, rb_ps, a2, a2g, es[NG - 2], es[NG - 1], h, nsl)
                y_sb = y_pool.tile([P, NB // P, DIM], F32, tag="y")
                pending_proj = [make_proj(nq, t, y_sb) for t in range(NB // P)]

            pending_tail()
            for fn in pending_proj:
                fn()

    nc.compile()
    return nc


def kernel(query, key, value, proj_w, proj_b):
    if "nc" not in _nc_cache:
        _nc_cache["nc"] = _build()
    nc = _nc_cache["nc"]

    scale = float(HD) ** -0.5
    wt_full = np.ascontiguousarray(proj_w.T.astype(np.float32))  # [in, out]
    in_maps = []
    for core in range(N_CORES):
        b, hg = divmod(core, N_CORES // B)
        sl = slice(hg * HG, (hg + 1) * HG)
        in_maps.append(
            {
                "qt": np.ascontiguousarray((query[b].T[sl] * scale), dtype=np.float16),
                "kt": np.ascontiguousarray(key[b].T[sl], dtype=np.float16),
                "v": np.ascontiguousarray(value[b][:, sl], dtype=np.float16),
                "wt": np.ascontiguousarray(wt_full[sl], dtype=np.float16),
            }
        )

    res = run_bass_kernel_spmd(nc, in_maps, list(range(N_CORES)))

    out = np.zeros((B, N, DIM), dtype=np.float32)
    for core in range(N_CORES):
        b = core // (N_CORES // B)
        out[b] += res.results[core]["out"]
    out += proj_b.astype(np.float32)
    return out


# revision 27
# speedup vs baseline: 1.1742x; 1.1742x over previous
"""Multi-head attention + output projection on 8 Trainium2 NeuronCores.

Problem (hardcoded): B=2, N=S=2048, DIM=1024, 8 heads, head_dim=128, fp32.
  out = softmax(Q K^T / sqrt(128)) V  -> reshape -> @ proj_w.T + proj_b

Sharding: data parallel on batch (2) x tensor parallel on heads (4 groups of
2 heads).  Each core computes attention for its 2 heads plus the partial
output projection restricted to its heads' columns; the host sums the 4
partial projections per batch and adds the bias.

Per-core kernel (matmul operands fp16, accumulation fp32 PSUM):
  S^T = K @ Q^T per 128-row s-chunk with s on partitions (softmax needs no
  on-chip transpose of P); exp on ScalarE (PSUM->SBUF, scale pre-applied to
  Q on host); out^T = V^T @ expS^T accumulated in PSUM.  Row sums feed an
  all-ones [128x128] matmul that colsums over partitions with the result
  broadcast to all 128 rows; reciprocal_approx_fast + multiply normalizes.

  The emission order is software-pipelined for the in-order engine queues:
  PV lags one group behind QK/exp, and each head's tail work (last PV,
  rowsum-tail matmuls, reciprocal, normalize) plus each block's projection
  are deferred into the NEXT head's stream so the exp pipeline on ScalarE
  never drains at head boundaries.
"""

import sys

sys.path.insert(0, "/opt/trn_rl_repo")

import numpy as np

import concourse.bass as bass  # noqa: F401  (engine namespaces live on nc)
import concourse.mybir as mybir
import concourse.tile as tile
from concourse import bacc
from concourse.bass_utils import run_bass_kernel_spmd

B = 2
N = 2048
S = 2048
DIM = 1024
NUM_HEADS = 8
HD = 128
N_CORES = 8
HEADS_PER_CORE = 2  # 4-way head parallel x 2-way batch parallel
HG = DIM // (NUM_HEADS // HEADS_PER_CORE)  # 256 dims per core
P = 128
SC = S // P  # 16 s-chunks
NB = 512  # query-column block
NQ = N // NB
GC = 2  # s-chunks per exp group
NG = SC // GC  # 8 groups per (head, block)
F32 = mybir.dt.float32
F16 = mybir.dt.float16

_nc_cache = {}


def _build():
    nc = bacc.Bacc(None, target_bir_lowering=False, debug=False, num_devices=1)

    qt = nc.dram_tensor("qt", [HG, N], F16, kind="ExternalInput").ap()
    kt = nc.dram_tensor("kt", [HG, S], F16, kind="ExternalInput").ap()
    v = nc.dram_tensor("v", [S, HG], F16, kind="ExternalInput").ap()
    wt = nc.dram_tensor("wt", [HG, DIM], F16, kind="ExternalInput").ap()
    out = nc.dram_tensor("out", [N, DIM], F32, kind="ExternalOutput").ap()

    EXPF = mybir.ActivationFunctionType.Exp

    with tile.TileContext(nc) as tc:
        with (
            tc.tile_pool(name="persist", bufs=1) as persist,
            tc.tile_pool(name="e_pool", bufs=8) as e_pool,
            tc.tile_pool(name="a_pool", bufs=3) as a_pool,
            tc.tile_pool(name="small", bufs=3) as small,
            tc.tile_pool(name="y_pool", bufs=2) as y_pool,
            tc.tile_pool(name="s_ps_pool", bufs=2, space="PSUM") as s_ps_pool,
            tc.tile_pool(name="acc_ps_pool", bufs=4, space="PSUM") as acc_ps_pool,
        ):
            # Resident inputs, sliced so the first QK group starts early.
            qt_sb = persist.tile([P, HEADS_PER_CORE, N], F16)
            kt_sb = persist.tile([P, HEADS_PER_CORE, S], F16)
            v_sb = persist.tile([P, HEADS_PER_CORE, SC, HD], F16)
            wt_sb = persist.tile([P, HEADS_PER_CORE, DIM], F16)
            qt_r = qt.rearrange("(h p) n -> p h n", p=P)
            kt_r = kt.rearrange("(h p) s -> p h s", p=P)
            v_r = v.rearrange("(c p) (h d) -> p h c d", p=P, h=HEADS_PER_CORE)
            wt_r = wt.rearrange("(h p) o -> p h o", p=P)
            ones_dram = nc.inline_tensor(np.ones((P, P), np.float16), name="ones_const")
            ones_mat = persist.tile([P, P], F16)
            nc.sync.dma_start(out=qt_sb[:, 0, 0:NB], in_=qt_r[:, 0, 0:NB])
            nc.sync.dma_start(out=kt_sb[:, 0, 0 : S // 4], in_=kt_r[:, 0, 0 : S // 4])
            nc.sync.dma_start(out=kt_sb[:, 0, S // 4 : S // 2], in_=kt_r[:, 0, S // 4 : S // 2])
            nc.sync.dma_start(out=v_sb[:, 0, 0 : SC // 2], in_=v_r[:, 0, 0 : SC // 2])
            nc.sync.dma_start(out=kt_sb[:, 0, S // 2 :], in_=kt_r[:, 0, S // 2 :])
            nc.sync.dma_start(out=v_sb[:, 0, SC // 2 :], in_=v_r[:, 0, SC // 2 :])
            nc.sync.dma_start(out=ones_mat, in_=ones_dram.ap())
            nc.sync.dma_start(out=qt_sb[:, 0, NB:], in_=qt_r[:, 0, NB:])
            nc.sync.dma_start(out=kt_sb[:, 1], in_=kt_r[:, 1])
            nc.sync.dma_start(out=qt_sb[:, 1], in_=qt_r[:, 1])
            nc.sync.dma_start(out=v_sb[:, 1], in_=v_r[:, 1])
            nc.sync.dma_start(out=wt_sb, in_=wt_r)

            # X^T: normalized attention outputs, head-dim on partitions.
            xt_sb = persist.tile([P, HEADS_PER_CORE, N], F16)

            def make_tail(o_ps, rb_ps, a2, a2g, e6, e7, h, nsl):
                def tail():
                    # last PV pair (group 7 only; group 6 ran in the main loop)
                    for j in range(GC):
                        si = GC * (NG - 1) + j
                        nc.tensor.matmul(
                            o_ps, v_sb[:, h, si, :], e7[:, j, :],
                            start=False, stop=(si == SC - 1),
                        )
                    with nc.allow_low_precision(reason="fp16 rowsum partials"):
                        nc.vector.tensor_add(a2g, a2g, e7)
                        nc.vector.tensor_add(a2, a2, a2g)
                    # rowsum: a2 halves now cover all 16 chunks
                    parts = [a2[:, 0, :], a2[:, 1, :]]
                    for pi, part in enumerate(parts):
                        nc.tensor.matmul(
                            rb_ps, ones_mat, part,
                            start=(pi == 0), stop=(pi == len(parts) - 1),
                        )
                    recip = small.tile([P, NB], F32, tag="recip")
                    nc.vector.reciprocal_approx_fast(out=recip, in_=rb_ps)
                    with nc.allow_low_precision(reason="fp16 attention output grid"):
                        nc.vector.tensor_mul(xt_sb[:, h, nsl], o_ps, recip)
                return tail

            def make_proj(nq, t, y_sb):
                def proj():
                    nt = nq * (NB // P) + t
                    for ot in range(2):
                        y_ps = acc_ps_pool.tile([P, NB], F32, tag="acc")
                        for h in range(HEADS_PER_CORE):
                            nc.tensor.matmul(
                                y_ps,
                                xt_sb[:, h, nt * P : (nt + 1) * P],
                                wt_sb[:, h, ot * NB : (ot + 1) * NB],
                                start=(h == 0),
                                stop=(h == HEADS_PER_CORE - 1),
                            )
                        nc.vector.tensor_copy(y_sb[:, t, ot * NB : (ot + 1) * NB], y_ps)
                    if t == NB // P - 1:
                        nc.sync.dma_start(
                            out=out[nq * NB : (nq + 1) * NB, :].rearrange(
                                "(t p) o -> p t o", p=P
                            ),
                            in_=y_sb,
                        )
                return proj

            pending_tail = None
            pending_proj = []
            for nq in range(NQ):
                nsl = slice(nq * NB, (nq + 1) * NB)
                for h in range(HEADS_PER_CORE):
                    q_blk = qt_sb[:, h, nsl]
                    o_ps = acc_ps_pool.tile([P, NB], F32, tag="acc")
                    a2 = a_pool.tile([P, GC, NB], F16, tag="a2")
                    a2g = a_pool.tile([P, GC, NB], F16, tag="a2g")
                    rb_ps = None
                    es = []  # exp tiles in flight
                    for g in range(NG):
                        s_ps = s_ps_pool.tile([P, GC, NB], F32, tag="s")
                        for j in range(GC):
                            si = GC * g + j
                            nc.tensor.matmul(
                                s_ps[:, j, :],
                                kt_sb[:, h, si * P : (si + 1) * P],
                                q_blk,
                                start=True,
                                stop=True,
                            )
                        e_t = e_pool.tile([P, GC, NB], F16, tag="e")
                        nc.scalar.activation(out=e_t, in_=s_ps, func=EXPF)
                        es.append(e_t)

                        if g == 0 and pending_tail is not None:
                            pending_tail()
                            pending_tail = None
                        if 4 <= g <= 7 and pending_proj:
                            pending_proj.pop(0)()

                        # PV + rowsum accumulation lag one group behind exp
                        if g >= 1:
                            pg = g - 1
                            pe = es[pg]
                            for j in range(GC):
                                si = GC * pg + j
                                nc.tensor.matmul(
                                    o_ps, v_sb[:, h, si, :], pe[:, j, :],
                                    start=(si == 0), stop=False,
                                )
                            with nc.allow_low_precision(
                                reason="fp16 rowsum partials; r ~2e3, ~3e-4 rel"
                            ):
                                if pg in (2, 3):
                                    acc = a2 if pg == 2 else a2g
                                    nc.vector.tensor_add(acc, es[pg - 2], pe)
                                elif pg >= 4:
                                    acc = a2 if pg % 2 == 0 else a2g
                                    nc.vector.tensor_add(acc, acc, pe)
                        if g == NG - 1:
                            rb_ps = acc_ps_pool.tile([P, NB], F32, tag="acc")
                    # defer: PV of group 7 + rowsum tail + normalize
                    pending_tail = make_tail(o_ps, rb_ps, a2, a2g, es[NG - 2], es[NG - 1], h, nsl)
                y_sb = y_pool.tile([P, NB // P, DIM], F32, tag="y")
                pending_proj = [make_proj(nq, t, y_sb) for t in range(NB // P)]

            pending_tail()
            for fn in pending_proj:
                fn()

    nc.compile()
    return nc


def kernel(query, key, value, proj_w, proj_b):
    if "nc" not in _nc_cache:
        _nc_cache["nc"] = _build()
    nc = _nc_cache["nc"]

    scale = float(HD) ** -0.5
    wt_full = np.ascontiguousarray(proj_w.T.astype(np.float32))  # [in, out]
    in_maps = []
    for core in range(N_CORES):
        b, hg = divmod(core, N_CORES // B)
        sl = slice(hg * HG, (hg + 1) * HG)
        in_maps.append(
            {
                "qt": np.ascontiguousarray((query[b].T[sl] * scale), dtype=np.float16),
                "kt": np.ascontiguousarray(key[b].T[sl], dtype=np.float16),
                "v": np.ascontiguousarray(value[b][:, sl], dtype=np.float16),
                "wt": np.ascontiguousarray(wt_full[sl], dtype=np.float16),
            }
        )

    res = run_bass_kernel_spmd(nc, in_maps, list(range(N_CORES)))

    out = np.zeros((B, N, DIM), dtype=np.float32)
    for core in range(N_CORES):
        b = core // (N_CORES // B)
        out[b] += res.results[core]["out"]
    out += proj_b.astype(np.float32)
    return out


# revision 28
# speedup vs baseline: 1.1958x; 1.0185x over previous
"""Multi-head attention + output projection on 8 Trainium2 NeuronCores.

Problem (hardcoded): B=2, N=S=2048, DIM=1024, 8 heads, head_dim=128, fp32.
  out = softmax(Q K^T / sqrt(128)) V  -> reshape -> @ proj_w.T + proj_b

Sharding: data parallel on batch (2) x tensor parallel on heads (4 groups of
2 heads).  Each core computes attention for its 2 heads plus the partial
output projection restricted to its heads' columns; the host sums the 4
partial projections per batch and adds the bias.

Per-core kernel (matmul operands fp16, accumulation fp32 PSUM):
  S^T = K @ Q^T per 128-row s-chunk with s on partitions (softmax needs no
  on-chip transpose of P); exp on ScalarE (PSUM->SBUF, scale pre-applied to
  Q on host); out^T = V^T @ expS^T accumulated in PSUM.  Row sums feed an
  all-ones [128x128] matmul that colsums over partitions with the result
  broadcast to all 128 rows; reciprocal_approx_fast + multiply normalizes.

  The emission order is software-pipelined for the in-order engine queues:
  PV lags one group behind QK/exp, and each head's tail work (last PV,
  rowsum-tail matmuls, reciprocal, normalize) plus each block's projection
  are deferred into the NEXT head's stream so the exp pipeline on ScalarE
  never drains at head boundaries.
"""

import sys

sys.path.insert(0, "/opt/trn_rl_repo")

import numpy as np

import concourse.bass as bass  # noqa: F401  (engine namespaces live on nc)
import concourse.mybir as mybir
import concourse.tile as tile
from concourse import bacc
from concourse.bass_utils import run_bass_kernel_spmd

B = 2
N = 2048
S = 2048
DIM = 1024
NUM_HEADS = 8
HD = 128
N_CORES = 8
HEADS_PER_CORE = 2  # 4-way head parallel x 2-way batch parallel
HG = DIM // (NUM_HEADS // HEADS_PER_CORE)  # 256 dims per core
P = 128
SC = S // P  # 16 s-chunks
NB = 512  # query-column block
NQ = N // NB
GC = 2  # s-chunks per exp group
NG = SC // GC  # 8 groups per (head, block)
F32 = mybir.dt.float32
F16 = mybir.dt.float16

_nc_cache = {}


def _build():
    nc = bacc.Bacc(None, target_bir_lowering=False, debug=False, num_devices=1)

    qt = nc.dram_tensor("qt", [HG, N], F16, kind="ExternalInput").ap()
    kt = nc.dram_tensor("kt", [HG, S], F16, kind="ExternalInput").ap()
    v = nc.dram_tensor("v", [S, HG], F16, kind="ExternalInput").ap()
    wt = nc.dram_tensor("wt", [HG, DIM], F16, kind="ExternalInput").ap()
    out = nc.dram_tensor("out", [N, DIM], F32, kind="ExternalOutput").ap()

    EXPF = mybir.ActivationFunctionType.Exp

    with tile.TileContext(nc) as tc:
        with (
            tc.tile_pool(name="persist", bufs=1) as persist,
            tc.tile_pool(name="e_pool", bufs=8) as e_pool,
            tc.tile_pool(name="a_pool", bufs=3) as a_pool,
            tc.tile_pool(name="small", bufs=3) as small,
            tc.tile_pool(name="y_pool", bufs=2) as y_pool,
            tc.tile_pool(name="s_ps_pool", bufs=2, space="PSUM") as s_ps_pool,
            tc.tile_pool(name="acc_ps_pool", bufs=4, space="PSUM") as acc_ps_pool,
        ):
            # Resident inputs, sliced so the first QK group starts early.
            qt_sb = persist.tile([P, HEADS_PER_CORE, N], F16)
            kt_sb = persist.tile([P, HEADS_PER_CORE, S], F16)
            v_sb = persist.tile([P, HEADS_PER_CORE, SC, HD], F16)
            wt_sb = persist.tile([P, HEADS_PER_CORE, DIM], F16)
            qt_r = qt.rearrange("(h p) n -> p h n", p=P)
            kt_r = kt.rearrange("(h p) s -> p h s", p=P)
            v_r = v.rearrange("(c p) (h d) -> p h c d", p=P, h=HEADS_PER_CORE)
            wt_r = wt.rearrange("(h p) o -> p h o", p=P)
            ones_dram = nc.inline_tensor(np.ones((P, P), np.float16), name="ones_const")
            ones_mat = persist.tile([P, P], F16)
            nc.sync.dma_start(out=qt_sb[:, 0, 0:NB], in_=qt_r[:, 0, 0:NB])
            nc.sync.dma_start(out=kt_sb[:, 0, 0 : S // 4], in_=kt_r[:, 0, 0 : S // 4])
            nc.sync.dma_start(out=kt_sb[:, 0, S // 4 : S // 2], in_=kt_r[:, 0, S // 4 : S // 2])
            nc.sync.dma_start(out=v_sb[:, 0, 0 : SC // 2], in_=v_r[:, 0, 0 : SC // 2])
            nc.sync.dma_start(out=kt_sb[:, 0, S // 2 :], in_=kt_r[:, 0, S // 2 :])
            nc.sync.dma_start(out=v_sb[:, 0, SC // 2 :], in_=v_r[:, 0, SC // 2 :])
            nc.sync.dma_start(out=ones_mat, in_=ones_dram.ap())
            nc.sync.dma_start(out=qt_sb[:, 0, NB:], in_=qt_r[:, 0, NB:])
            nc.sync.dma_start(out=kt_sb[:, 1], in_=kt_r[:, 1])
            nc.sync.dma_start(out=qt_sb[:, 1], in_=qt_r[:, 1])
            nc.sync.dma_start(out=v_sb[:, 1], in_=v_r[:, 1])
            nc.sync.dma_start(out=wt_sb, in_=wt_r)

            # X^T: normalized attention outputs, head-dim on partitions.
            xt_sb = persist.tile([P, HEADS_PER_CORE, N], F16)

            def make_tail(o_ps, rb_ps, a2, a2g, e6, e7, h, nsl):
                def tail():
                    # last PV pair (group 7 only; group 6 ran in the main loop)
                    for j in range(GC):
                        si = GC * (NG - 1) + j
                        nc.tensor.matmul(
                            o_ps, v_sb[:, h, si, :], e7[:, j, :],
                            start=False, stop=(si == SC - 1),
                        )
                    with nc.allow_low_precision(reason="fp16 rowsum partials"):
                        nc.vector.tensor_add(a2g, a2g, e7)
                        nc.vector.tensor_add(a2, a2, a2g)
                    # rowsum: a2 halves now cover all 16 chunks
                    parts = [a2[:, 0, :], a2[:, 1, :]]
                    for pi, part in enumerate(parts):
                        nc.tensor.matmul(
                            rb_ps, ones_mat, part,
                            start=(pi == 0), stop=(pi == len(parts) - 1),
                        )
                    recip = small.tile([P, NB], F32, tag="recip")
                    nc.vector.reciprocal_approx_fast(out=recip, in_=rb_ps)
                    with nc.allow_low_precision(reason="fp16 attention output grid"):
                        nc.vector.tensor_mul(xt_sb[:, h, nsl], o_ps, recip)
                return tail

            def make_proj(nq, t, y_sb):
                def proj():
                    nt = nq * (NB // P) + t
                    for ot in range(2):
                        y_ps = acc_ps_pool.tile([P, NB], F32, tag="acc")
                        for h in range(HEADS_PER_CORE):
                            nc.tensor.matmul(
                                y_ps,
                                xt_sb[:, h, nt * P : (nt + 1) * P],
                                wt_sb[:, h, ot * NB : (ot + 1) * NB],
                                start=(h == 0),
                                stop=(h == HEADS_PER_CORE - 1),
                            )
                        nc.vector.tensor_copy(y_sb[:, t, ot * NB : (ot + 1) * NB], y_ps)
                    if t in (1, NB // P - 1):
                        lo = 0 if t == 1 else 2
                        nc.sync.dma_start(
                            out=out[nq * NB + lo * P : nq * NB + (t + 1) * P, :].rearrange(
                                "(t p) o -> p t o", p=P
                            ),
                            in_=y_sb[:, lo : t + 1, :],
                        )
                return proj

            pending_tail = None
            pending_proj = []
            for nq in range(NQ):
                nsl = slice(nq * NB, (nq + 1) * NB)
                for h in range(HEADS_PER_CORE):
                    q_blk = qt_sb[:, h, nsl]
                    o_ps = acc_ps_pool.tile([P, NB], F32, tag="acc")
                    a2 = a_pool.tile([P, GC, NB], F16, tag="a2")
                    a2g = a_pool.tile([P, GC, NB], F16, tag="a2g")
                    rb_ps = None
                    es = []  # exp tiles in flight
                    for g in range(NG):
                        s_ps = s_ps_pool.tile([P, GC, NB], F32, tag="s")
                        for j in range(GC):
                            si = GC * g + j
                            nc.tensor.matmul(
                                s_ps[:, j, :],
                                kt_sb[:, h, si * P : (si + 1) * P],
                                q_blk,
                                start=True,
                                stop=True,
                            )
                        e_t = e_pool.tile([P, GC, NB], F16, tag="e")
                        nc.scalar.activation(out=e_t, in_=s_ps, func=EXPF)
                        es.append(e_t)

                        if g == 0 and pending_tail is not None:
                            pending_tail()
                            pending_tail = None
                        if 4 <= g <= 7 and pending_proj:
                            pending_proj.pop(0)()

                        # PV + rowsum accumulation lag one group behind exp
                        if g >= 1:
                            pg = g - 1
                            pe = es[pg]
                            for j in range(GC):
                                si = GC * pg + j
                                nc.tensor.matmul(
                                    o_ps, v_sb[:, h, si, :], pe[:, j, :],
                                    start=(si == 0), stop=False,
                                )
                            with nc.allow_low_precision(
                                reason="fp16 rowsum partials; r ~2e3, ~3e-4 rel"
                            ):
                                if pg in (2, 3):
                                    acc = a2 if pg == 2 else a2g
                                    nc.vector.tensor_add(acc, es[pg - 2], pe)
                                elif pg >= 4:
                                    acc = a2 if pg % 2 == 0 else a2g
                                    nc.vector.tensor_add(acc, acc, pe)
                        if g == NG - 1:
                            rb_ps = acc_ps_pool.tile([P, NB], F32, tag="acc")
                    # defer: PV of group 7 + rowsum tail + normalize
                    pending_tail = make_tail(o_ps, rb_ps, a2, a2g, es[NG - 2], es[NG - 1], h, nsl)
                y_sb = y_pool.tile([P, NB // P, DIM], F32, tag="y")
                pending_proj = [make_proj(nq, t, y_sb) for t in range(NB // P)]

            pending_tail()
            for fn in pending_proj:
                fn()

    nc.compile()
    return nc


def kernel(query, key, value, proj_w, proj_b):
    if "nc" not in _nc_cache:
        _nc_cache["nc"] = _build()
    nc = _nc_cache["nc"]

    scale = float(HD) ** -0.5
    wt_full = np.ascontiguousarray(proj_w.T.astype(np.float32))  # [in, out]
    in_maps = []
    for core in range(N_CORES):
        b, hg = divmod(core, N_CORES // B)
        sl = slice(hg * HG, (hg + 1) * HG)
        in_maps.append(
            {
                "qt": np.ascontiguousarray((query[b].T[sl] * scale), dtype=np.float16),
                "kt": np.ascontiguousarray(key[b].T[sl], dtype=np.float16),
                "v": np.ascontiguousarray(value[b][:, sl], dtype=np.float16),
                "wt": np.ascontiguousarray(wt_full[sl], dtype=np.float16),
            }
        )

    res = run_bass_kernel_spmd(nc, in_maps, list(range(N_CORES)))

    out = np.zeros((B, N, DIM), dtype=np.float32)
    for core in range(N_CORES):
        b = core // (N_CORES // B)
        out[b] += res.results[core]["out"]
    out += proj_b.astype(np.float32)
    return out
